# revision 1
# baseline (speedup 1.0000x reference)
"""Multi-head causal attention (B=8, S=1024, C=1024, H=16, dk=dv=64) on 8 trn2 cores.

Sharding: data-parallel over batch. Each NeuronCore processes one batch element
end-to-end (projections + attention + output projection); no collectives.

Per-core layout:
  inputs (host-prepped): xq/xk/xv = X^T [C, S] bf16, packed weights
  wq/wk [C, H*DK] (wq pre-scaled by 1/sqrt(dk)), wv [C, H*DV], wo [H*DV, C],
  biases in per-partition / replicated layouts.

  QT = wq.T @ xq  -> [H*DK, S]   (head-major rows)
  KT = wk.T @ xk  -> [H*DK, S]
  V  = xv.T @ wv  -> [S, H*DV]   (+ appended ones column per head)
  per head h, q-chunk: St[t, q] = KT_h.T-contract -> exp -> mask ->
    O^T/r accumulated via matmul(lhsT=[V_h | 1], rhs=P)  (row 64 = softmax denom)
  Y = concat(O)^T-contract @ wo + bo -> [S, C] f32
"""

import math
import os
import sys

import numpy as np

try:
    import concourse.bass as bass
except ImportError:  # make concourse importable in a bare grading dir
    for _p in ("/opt/trn_rl_repo", os.path.expanduser("~/.axon_site/_ro/trn_rl_repo")):
        if os.path.isdir(_p) and _p not in sys.path:
            sys.path.insert(0, _p)
    import concourse.bass as bass

from contextlib import ExitStack

import ml_dtypes

import concourse.mybir as mybir
import concourse.tile as tile
from concourse import bacc
from concourse.bass_utils import run_bass_kernel_spmd

def _setup_act_tables():
    """Pin the ACT function table to the set that covers exp+ln+identity+copy
    so the kernel never reloads LUTs mid-flight. Both the bacc-side pass and
    walrus must see the same (reordered) act_info.json."""
    import json
    import shutil
    import tempfile

    import concourse.hw_specs as hw_specs
    from concourse import bacc as _bacc

    if os.environ.get("BASS_ACT_ROOT_JSON_PATH"):
        return  # already configured
    from neuronxcc.driver.Job import Job

    orig = os.path.join(
        Job.getPackageDir(), "pwp", "pwp_bin_trainium", "act_info.json"
    )
    assert os.path.isfile(orig), orig
    dst = os.path.join(tempfile.gettempdir(), "mha_act_tables")
    if not os.path.isdir(dst):
        tmp = dst + ".tmp"
        shutil.rmtree(tmp, ignore_errors=True)
        shutil.copytree(os.path.dirname(orig), tmp)
        with open(os.path.join(tmp, "act_info.json")) as f:
            info = json.load(f)
        sets = info["act_func_sets"]
        want = [s for s in sets if s["name"] == "natural_log_exp_and_others"]
        rest = [s for s in sets if s["name"] != "natural_log_exp_and_others"]
        info["act_func_sets"] = want + rest
        with open(os.path.join(tmp, "act_info.json"), "w") as f:
            json.dump(info, f)
        os.replace(tmp, dst)
    path = os.path.join(dst, "act_info.json")
    os.environ["BASS_ACT_ROOT_JSON_PATH"] = path

    def patched(module_arch):
        with open(path) as af:
            act_info = json.load(af)
        return {
            ent["name"]: {
                mybir.ActivationFunctionType.from_pwp(v) for v in ent["act"].keys()
            }
            for ent in act_info["act_func_sets"]
        }

    hw_specs.get_activation_tables = patched
    _bacc.get_activation_tables = patched
    from concourse import bass_interp as _bi

    _bi.get_activation_tables = patched


B, S, C = 8, 1024, 1024
H, DK, DV = 16, 64, 64
P = 128
NT = 8  # number of 128-tiles along S / C / H*DK
CH = 512  # free-dim chunk (one PSUM bank of fp32)
NCH = S // CH


FP = mybir.dt.float32
BF = mybir.dt.bfloat16
BF_NP = ml_dtypes.bfloat16
AFT = mybir.ActivationFunctionType
ALU = mybir.AluOpType


def build_nc() -> bass.Bass:
    _setup_act_tables()
    nc = bacc.Bacc()

    xq = nc.dram_tensor("xq", [C, S], BF, kind="ExternalInput")
    xk = nc.dram_tensor("xk", [C, S], BF, kind="ExternalInput")
    xv = nc.dram_tensor("xv", [C, S], BF, kind="ExternalInput")
    wq = nc.dram_tensor("wq", [C, H * DK], BF, kind="ExternalInput")
    wk = nc.dram_tensor("wk", [C, H * DK], BF, kind="ExternalInput")
    wv = nc.dram_tensor("wv", [C, H * DV], BF, kind="ExternalInput")
    wo = nc.dram_tensor("wo", [H * DV, C], BF, kind="ExternalInput")
    bqd = nc.dram_tensor("bq", [P, NT], FP, kind="ExternalInput")
    bkd = nc.dram_tensor("bk", [P, NT], FP, kind="ExternalInput")
    bvd = nc.dram_tensor("bv", [P, H * DV], FP, kind="ExternalInput")
    bod = nc.dram_tensor("bo", [1, C], FP, kind="ExternalInput")
    y = nc.dram_tensor("y", [S, C], FP, kind="ExternalOutput")

    # constants baked into the NEFF, packed into one tensor (single DMA)
    # cols 0:128   = additive causal mask for [t,q] diag block (0 if t<=q else -big)
    # row 0, cols 128:192 = ones[1,64]   (denominator replicate lhsT)
    # row 0, cols 192:320 = ones[1,128]  (output-bias matmul lhsT)
    cblk_np = np.zeros((P, 384), np.float32)
    cblk_np[0, P : P + DV] = 1.0
    cblk_np[0, P + DV : P + DV + P] = 1.0
    cblk_np[0, P + DV + P : P + 2 * DV + P] = -1.0
    cblk_d = nc.inline_tensor(cblk_np, "cblk")
    # binary causal mask [t,q] (1 iff t<=q), bf16, multiplied post-exp
    tri_d = nc.inline_tensor(
        np.triu(np.ones((P, P), np.float32)).astype(BF_NP), "tri"
    )

    xq_r = xq.rearrange("(ko p) s -> p ko s", p=P)
    xk_r = xk.rearrange("(ko p) s -> p ko s", p=P)
    xv_r = xv.rearrange("(ko p) s -> p ko s", p=P)
    wq_r = wq.rearrange("(ko p) m -> p ko m", p=P)
    wk_r = wk.rearrange("(ko p) m -> p ko m", p=P)
    wv_r = wv.rearrange("(ko p) m -> p ko m", p=P)
    wo_r = wo.rearrange("(ko p) c -> p ko c", p=P)
    y_r = y.rearrange("(mo p) c -> p mo c", p=P)

    with tile.TileContext(nc) as tc, ExitStack() as octx:
        const = octx.enter_context(tc.tile_pool(name="const", bufs=1))
        qk = octx.enter_context(tc.tile_pool(name="qk", bufs=1))
        opool = octx.enter_context(tc.tile_pool(name="oT", bufs=1))
        ppool = octx.enter_context(tc.tile_pool(name="p", bufs=8))
        small = octx.enter_context(tc.tile_pool(name="small", bufs=2))
        ypool = octx.enter_context(tc.tile_pool(name="y", bufs=2))

        cblk_sb = const.tile([P, 384], FP, tag="cblk")
        nc.sync.dma_start(cblk_sb, cblk_d[:])
        ones64_sb = cblk_sb[0:1, P : P + DV]
        ones128_sb = cblk_sb[0:1, P + DV : P + DV + P]
        negones64_sb = cblk_sb[0:1, P + DV + P : P + 2 * DV + P]
        tri_sb = const.tile([P, P], BF, tag="tri")
        nc.sync.dma_start(tri_sb, tri_d[:])
        bq_sb = const.tile([P, NT], FP, tag="bq")
        nc.sync.dma_start(bq_sb, bqd[:])
        bk_sb = const.tile([P, NT], FP, tag="bk")
        nc.sync.dma_start(bk_sb, bkd[:])
        bo_sb = const.tile([1, C], FP, tag="bo")
        nc.sync.dma_start(bo_sb, bod[:])
        borep_sb = const.tile([P, C], FP, tag="borep")
        nc.gpsimd.partition_broadcast(borep_sb, bo_sb)

        qT_sb = qk.tile([P, NT, S], BF, tag="qT")
        kT2_sb = qk.tile([P, H, S], BF, tag="kT2")
        v_sb = qk.tile([P, NT, H, DV + 1], BF, tag="v")
        oT_sb = opool.tile([P, NT, S], BF, tag="oT")

        nc.vector.memset(v_sb[:, :, :, DV], 1.0)
        # zero the unused half of each head's K^T tile so St matmuls can
        # contract over the full 128 partitions (keeps the PE array fully
        # utilized; the zero lhsT rows nullify the other head's Q rows)
        for h in range(H):
            hz = DK if h % 2 == 0 else 0
            nc.gpsimd.memset(kT2_sb[hz : hz + DK, h, :], 0.0)

        # ---------------- projections ----------------
        with ExitStack() as ictx:
            wpool = ictx.enter_context(tc.tile_pool(name="wqkv", bufs=2))
            xpool = ictx.enter_context(tc.tile_pool(name="xin", bufs=2))
            bv_sb = wpool.tile([P, H * DV], FP, tag="bv", name="bv_sb")
            nc.sync.dma_start(bv_sb, bvd[:])
            psproj = ictx.enter_context(
                tc.tile_pool(name="psproj", bufs=8, space="PSUM")
            )

            # Q^T and K^T: out[hk, s] ; lhsT = w tile [c, hk], rhs = x^T [c, s]
            for x_r, w_r, b_sb, out_sb in (
                (xq_r, wq_r, bq_sb, qT_sb),
                (xk_r, wk_r, bk_sb, None),
            ):
                w_sb = wpool.tile([P, NT, H * DK], BF, tag="w", name="w_sb")
                x_sb = xpool.tile([P, NT, S], BF, tag="x", name="x_sb")
                for kc in range(NT):
                    nc.sync.dma_start(w_sb[:, kc], w_r[:, kc])
                    nc.sync.dma_start(x_sb[:, kc], x_r[:, kc])
                for n in range(NCH):
                    psums = []
                    for kc in range(NT):
                        for m in range(NT):
                            if kc == 0:
                                psums.append(psproj.tile([P, CH], FP, tag="proj", name=f"proj_ps_{m}"))
                            nc.tensor.matmul(
                                psums[m],
                                w_sb[:, kc, m * P : (m + 1) * P],
                                x_sb[:, kc, n * CH : (n + 1) * CH],
                                start=(kc == 0),
                                stop=(kc == NT - 1),
                            )
                    for m in range(NT):
                        if out_sb is not None:
                            nc.vector.tensor_scalar_add(
                                out_sb[:, m, n * CH : (n + 1) * CH],
                                psums[m],
                                b_sb[:, m : m + 1],
                            )
                        else:
                            # K^T: split the head pair into per-head tiles
                            sl = slice(n * CH, (n + 1) * CH)
                            nc.vector.tensor_scalar_add(
                                kT2_sb[0:DK, 2 * m, sl],
                                psums[m][0:DK],
                                b_sb[0:DK, m : m + 1],
                            )
                            nc.vector.tensor_scalar_add(
                                kT2_sb[DK:P, 2 * m + 1, sl],
                                psums[m][DK:P],
                                b_sb[DK:P, m : m + 1],
                            )

            # V: out[s, hv] ; lhsT = x^T tile [c, s], rhs = wv [c, hv]
            wv_sb = wpool.tile([P, NT, H * DV], BF, tag="w", name="wv_sb")
            nc.sync.dma_start(wv_sb, wv_r)
            xv_sb = xpool.tile([P, NT, S], BF, tag="x", name="xv_sb")
            nc.sync.dma_start(xv_sb, xv_r)
            for n in range(NCH):
                psums = []
                for kc in range(NT):
                    for m in range(NT):
                        if kc == 0:
                            psums.append(psproj.tile([P, CH], FP, tag="proj", name=f"proj_ps_{m}"))
                        nc.tensor.matmul(
                            psums[m],
                            xv_sb[:, kc, m * P : (m + 1) * P],
                            wv_sb[:, kc, n * CH : (n + 1) * CH],
                            start=(kc == 0),
                            stop=(kc == NT - 1),
                        )
                for m in range(NT):
                    dst = v_sb[:, m, 8 * n : 8 * (n + 1), 0:DV]
                    nc.vector.tensor_tensor(
                        dst,
                        psums[m].rearrange("p (h v) -> p h v", v=DV),
                        bv_sb[:, n * CH : (n + 1) * CH].rearrange(
                            "p (h v) -> p h v", v=DV
                        ),
                        ALU.add,
                    )

        # wo loads into the space freed by the projection pools
        wopool = octx.enter_context(tc.tile_pool(name="wo", bufs=1))
        wo_sb = wopool.tile([P, NT, C], BF, tag="wo")
        nc.sync.dma_start(wo_sb, wo_r)

        # ---------------- attention + interleaved output projection ------
        # Head pairs (2h, 2h+1) sit on partitions 0:64 / 64:128 of the same
        # qT/kT tile, so their K=64 St matmuls land on disjoint PE row-groups
        # and run concurrently when issued back-to-back. Normalization is
        # deferred: O^T is evacuated unnormalized with the denominator row
        # collected per q-chunk round, then one batched reciprocal rescales
        # all 16 heads at once, and the output projection for that q-chunk
        # follows immediately (dense matmuls that keep the PE warm).
        ps_st = octx.enter_context(tc.tile_pool(name="ps_st", bufs=4, space="PSUM"))
        ps_o = octx.enter_context(tc.tile_pool(name="ps_o", bufs=2, space="PSUM"))
        ps_misc = octx.enter_context(tc.tile_pool(name="ps_misc", bufs=2, space="PSUM"))
        oupool = octx.enter_context(tc.tile_pool(name="ou", bufs=3))

        def attn_group(hp, jc, heads=(0, 1)):
            pos = {}
            i_list = list(range(0, min(NT, 4 * jc + 4)))
            for sub in heads:
                pos[sub] = ps_o.tile(
                    [P, CH], FP, tag="o", name=f"po_{hp}_{jc}_{sub}"
                )[: DV + 1]
            for idx, i in enumerate(i_list):
                # valid q columns in this chunk start at the diagonal
                off = max(0, i * P - jc * CH)
                w = CH - off
                pchs = {}
                for sub in heads:
                    hm = sub * DK
                    pst = ps_st.tile(
                        [P, CH], FP, tag="st", name=f"st_{hp}_{jc}_{i}_{sub}"
                    )[:, :w]
                    nc.tensor.matmul(
                        pst,
                        kT2_sb[:, 2 * hp + sub, i * P : (i + 1) * P],
                        qT_sb[:, hp, jc * CH + off : (jc + 1) * CH],
                        start=True,
                        stop=True,
                    )
                    pch = ppool.tile(
                        [P, CH], BF, tag="p", name=f"p_{hp}_{jc}_{i}_{sub}"
                    )[:, :w]
                    nc.scalar.activation(pch, pst, AFT.Exp)
                    if i * P >= jc * CH:
                        # diagonal block: zero the upper-left triangle (t>q)
                        nc.vector.tensor_tensor(
                            pch[:, 0:P], pch[:, 0:P], tri_sb, ALU.mult
                        )
                    pchs[sub] = pch
                for sub in heads:
                    nc.tensor.matmul(
                        pos[sub][:, off:],
                        v_sb[:, i, 2 * hp + sub, :],
                        pchs[sub],
                        start=(idx == 0),
                        stop=(idx == len(i_list) - 1),
                    )
            for sub in heads:
                hm = sub * DK
                po = pos[sub]
                # evacuate the whole accumulator (O^T rows + denominator row)
                # to SBUF at once so the PSUM bank recycles immediately; the
                # 1/r chain then runs off the critical path:
                # Ln/Exp(-x) on ACT rows, partition-broadcast on GpSimd,
                # one DVE multiply into oT.
                ou = oupool.tile([DV + 1, CH], FP, tag="ou", name=f"ou_{hp}_{jc}_{sub}")
                nc.vector.tensor_copy(out=ou, in_=po)
                rln = small.tile([1, CH], FP, tag="rln")
                nc.scalar.activation(rln, ou[DV : DV + 1, :], AFT.Ln)
                rrow = small.tile([1, CH], FP, tag="rrow")
                nc.scalar.activation(rrow, rln, AFT.Exp, scale=-1.0)
                rrep = small.tile([DV, CH], FP, tag="rrep")
                nc.gpsimd.partition_broadcast(rrep, rrow)
                nc.vector.tensor_tensor(
                    oT_sb[hm : hm + DV, hp, jc * CH : (jc + 1) * CH],
                    ou[:DV],
                    rrep,
                    ALU.mult,
                )

        def outproj_mtile(m):
            for n in range(NCH):
                py = ps_misc.tile([P, CH], FP, tag="misc", name=f"py_{m}_{n}")
                for kc in range(NT):
                    nc.tensor.matmul(
                        py,
                        oT_sb[:, kc, m * P : (m + 1) * P],
                        wo_sb[:, kc, n * CH : (n + 1) * CH],
                        start=(kc == 0),
                        stop=(kc == NT - 1),
                    )
                yt = ypool.tile([P, CH], FP, tag="y")
                nc.vector.tensor_tensor(
                    yt, py, borep_sb[:, n * CH : (n + 1) * CH], ALU.add
                )
                nc.sync.dma_start(y_r[:, m, n * CH : (n + 1) * CH], yt)

        for jc in range(NCH):
            for hp in range(H // 2):
                attn_group(hp, jc, heads=(0,))
                attn_group(hp, jc, heads=(1,))
            for m in range(4 * jc, 4 * jc + 4):
                outproj_mtile(m)

    nc.finalize()
    return nc


_NC_CACHE = None


def _get_nc() -> bass.Bass:
    global _NC_CACHE
    if _NC_CACHE is None:
        _NC_CACHE = build_nc()
    return _NC_CACHE


def prep_shared(Wq, bq, Wk, bk, Wv, bv, Wo, bo):
    """Host-side packing of weights/biases (shared by all cores)."""
    scale = 1.0 / math.sqrt(DK)
    Wq = np.asarray(Wq, np.float32)
    Wk = np.asarray(Wk, np.float32)
    Wv = np.asarray(Wv, np.float32)
    Wo = np.asarray(Wo, np.float32)
    out = {
        "wq": np.ascontiguousarray(
            (Wq.transpose(1, 0, 2).reshape(C, H * DK) * scale).astype(BF_NP)
        ),
        "wk": np.ascontiguousarray(
            Wk.transpose(1, 0, 2).reshape(C, H * DK).astype(BF_NP)
        ),
        "wv": np.ascontiguousarray(
            Wv.transpose(1, 0, 2).reshape(C, H * DV).astype(BF_NP)
        ),
        "wo": Wo.astype(BF_NP),
        "bq": np.ascontiguousarray(
            (np.asarray(bq, np.float32).reshape(H * DK) * scale)
            .reshape(NT, P)
            .T.astype(np.float32)
        ),
        "bk": np.ascontiguousarray(
            np.asarray(bk, np.float32).reshape(NT, P).T.astype(np.float32)
        ),
        "bv": np.ascontiguousarray(
            np.broadcast_to(
                np.asarray(bv, np.float32).reshape(1, H * DV), (P, H * DV)
            ).astype(np.float32)
        ),
        "bo": np.ascontiguousarray(np.asarray(bo, np.float32).reshape(1, C)),
    }
    return out


def prep_core(q_embs_b, k_embs_b, v_embs_b):
    return {
        "xq": np.ascontiguousarray(np.asarray(q_embs_b, np.float32).T.astype(BF_NP)),
        "xk": np.ascontiguousarray(np.asarray(k_embs_b, np.float32).T.astype(BF_NP)),
        "xv": np.ascontiguousarray(np.asarray(v_embs_b, np.float32).T.astype(BF_NP)),
    }


def kernel(q_embs, k_embs, v_embs, Wq, bq, Wk, bk, Wv, bv, Wo, bo, **run_kwargs):
    nc = _get_nc()
    shared = prep_shared(Wq, bq, Wk, bk, Wv, bv, Wo, bo)
    q_embs = np.asarray(q_embs, np.float32)
    k_embs = np.asarray(k_embs, np.float32)
    v_embs = np.asarray(v_embs, np.float32)
    in_maps = []
    for b in range(B):
        m = dict(shared)
        m.update(prep_core(q_embs[b], k_embs[b], v_embs[b]))
        in_maps.append(m)
    res = run_bass_kernel_spmd(nc, in_maps, core_ids=list(range(B)), **run_kwargs)
    out = np.stack([res.results[i]["y"] for i in range(B)], axis=0)
    if run_kwargs:
        kernel.last_results = res
    return out


if __name__ == "__main__":
    rng = np.random.default_rng(0)
    inputs = {
        "q_embs": rng.standard_normal((B, S, C), np.float32),
        "k_embs": rng.standard_normal((B, S, C), np.float32),
        "v_embs": rng.standard_normal((B, S, C), np.float32),
        "Wq": rng.standard_normal((H, C, DK), np.float32) * 0.02,
        "bq": np.zeros((H, DK), np.float32),
        "Wk": rng.standard_normal((H, C, DK), np.float32) * 0.02,
        "bk": np.zeros((H, DK), np.float32),
        "Wv": rng.standard_normal((H, C, DV), np.float32) * 0.02,
        "bv": np.zeros((H, DV), np.float32),
        "Wo": rng.standard_normal((H * DV, C), np.float32) * 0.02,
        "bo": np.zeros((C,), np.float32),
    }
    out = kernel(**inputs)
    print(out.shape, out.dtype)



# revision 17
# speedup vs baseline: 1.1089x; 1.1089x over previous
"""Multi-head causal attention (B=8, S=1024, C=1024, H=16, dk=dv=64) on 8 trn2 cores.

Sharding: data-parallel over batch. Each NeuronCore processes one batch element
end-to-end (projections + attention + output projection); no collectives.

v2: software-pipelined single-pass schedule. The K/Q/V projection matmul
groups are streamed *into* the attention phase (which is ACT/exp-bound) so the
PE never idles; weights arrive via just-in-time DMA of host-repacked
contiguous slices. The softmax denominator reciprocal moved from the ACT
(Ln/Exp chain) to a single DVE reciprocal_approx_fast reading PSUM directly.

Per-core math (all bf16 matmuls, fp32 PSUM):
  QT = wq.T @ xq  -> [H*DK, S]   (head-pair-major rows, pre-scaled 1/sqrt(dk))
  KT = wk.T @ xk  -> per-head [64, S] tiles zero-padded to 128 partitions
  V  = xv.T @ wv  -> [S, H*DV]   (+ ones column per head for the denominator)
  per (head pair, q-chunk): St[t,q] -> exp -> tri-mask -> O^T (+ denom row)
    via matmul(lhsT=[V_h | 1], rhs=P); normalize with DVE 1/r * broadcast
  Y = oT.T-contract @ wo + bo -> [S, C] f32
"""

import math
import os
import sys

import numpy as np

try:
    import concourse.bass as bass
except ImportError:  # make concourse importable in a bare grading dir
    for _p in ("/opt/trn_rl_repo", os.path.expanduser("~/.axon_site/_ro/trn_rl_repo")):
        if os.path.isdir(_p) and _p not in sys.path:
            sys.path.insert(0, _p)
    import concourse.bass as bass

from contextlib import ExitStack

import ml_dtypes

import concourse.mybir as mybir
import concourse.tile as tile
from concourse import bacc
from concourse.bass_utils import run_bass_kernel_spmd


def _setup_act_tables():
    """Pin the ACT function table to the set that covers exp+ln+identity+copy
    so the kernel never reloads LUTs mid-flight."""
    import json
    import shutil
    import tempfile

    import concourse.hw_specs as hw_specs
    from concourse import bacc as _bacc

    if os.environ.get("BASS_ACT_ROOT_JSON_PATH"):
        return  # already configured
    from neuronxcc.driver.Job import Job

    orig = os.path.join(
        Job.getPackageDir(), "pwp", "pwp_bin_trainium", "act_info.json"
    )
    assert os.path.isfile(orig), orig
    dst = os.path.join(tempfile.gettempdir(), "mha_act_tables")
    if not os.path.isdir(dst):
        tmp = dst + ".tmp"
        shutil.rmtree(tmp, ignore_errors=True)
        shutil.copytree(os.path.dirname(orig), tmp)
        with open(os.path.join(tmp, "act_info.json")) as f:
            info = json.load(f)
        sets = info["act_func_sets"]
        want = [s for s in sets if s["name"] == "natural_log_exp_and_others"]
        rest = [s for s in sets if s["name"] != "natural_log_exp_and_others"]
        info["act_func_sets"] = want + rest
        with open(os.path.join(tmp, "act_info.json"), "w") as f:
            json.dump(info, f)
        os.replace(tmp, dst)
    path = os.path.join(dst, "act_info.json")
    os.environ["BASS_ACT_ROOT_JSON_PATH"] = path

    def patched(module_arch):
        with open(path) as af:
            act_info = json.load(af)
        return {
            ent["name"]: {
                mybir.ActivationFunctionType.from_pwp(v) for v in ent["act"].keys()
            }
            for ent in act_info["act_func_sets"]
        }

    hw_specs.get_activation_tables = patched
    _bacc.get_activation_tables = patched
    from concourse import bass_interp as _bi

    _bi.get_activation_tables = patched


B, S, C = 8, 1024, 1024
H, DK, DV = 16, 64, 64
P = 128
NT = 8  # number of 128-tiles along S / C / H*DK
CH = 512  # free-dim chunk (one PSUM bank of fp32)
NCH = S // CH
NPAIR = H // 2

FP = mybir.dt.float32
BF = mybir.dt.bfloat16
BF_NP = ml_dtypes.bfloat16
AFT = mybir.ActivationFunctionType
ALU = mybir.AluOpType


def build_nc() -> bass.Bass:
    _setup_act_tables()
    nc = bacc.Bacc()

    # host-packed inputs; see prep_shared/prep_core for layouts
    xq = nc.dram_tensor("xq", [NCH, P, NT, CH], BF, kind="ExternalInput")
    xk = nc.dram_tensor("xk", [NCH, P, NT, CH], BF, kind="ExternalInput")
    xv = nc.dram_tensor("xv", [NT, P, NT, P], BF, kind="ExternalInput")
    wq = nc.dram_tensor("wq", [NT, P, NT, P], BF, kind="ExternalInput")
    wk = nc.dram_tensor("wk", [NT, P, NT, P], BF, kind="ExternalInput")
    wv = nc.dram_tensor("wv", [NCH, P, NT, CH], BF, kind="ExternalInput")
    wo = nc.dram_tensor("wo", [H * DV, C], BF, kind="ExternalInput")
    bqd = nc.dram_tensor("bq", [P, NT], FP, kind="ExternalInput")
    bkd = nc.dram_tensor("bk", [P, NT], FP, kind="ExternalInput")
    bvd = nc.dram_tensor("bv", [P, H * DV], BF, kind="ExternalInput")
    bod = nc.dram_tensor("bo", [1, C], BF, kind="ExternalInput")
    y = nc.dram_tensor("y", [S, C], FP, kind="ExternalOutput")

    # binary causal mask [t,q] for the 128-wide diagonal block (1 iff t<=q)
    tri_d = nc.inline_tensor(
        np.triu(np.ones((P, P), np.float32)).astype(BF_NP), "tri"
    )

    wo_r = wo.rearrange("(ko p) c -> p ko c", p=P)
    y_r = y.rearrange("(mo p) c -> p mo c", p=P)

    with tile.TileContext(nc) as tc, ExitStack() as octx:
        const = octx.enter_context(tc.tile_pool(name="const", bufs=1))
        big = octx.enter_context(tc.tile_pool(name="big", bufs=1))
        wkp = octx.enter_context(tc.tile_pool(name="wkp", bufs=3))
        wqp = octx.enter_context(tc.tile_pool(name="wqp", bufs=3))
        xvp = octx.enter_context(tc.tile_pool(name="xvp", bufs=3))
        pst = octx.enter_context(tc.tile_pool(name="pst", bufs=4, space="PSUM"))
        ppo = octx.enter_context(tc.tile_pool(name="ppo", bufs=2, space="PSUM"))
        ppy = octx.enter_context(tc.tile_pool(name="ppy", bufs=2, space="PSUM"))
        pchp = octx.enter_context(tc.tile_pool(name="pchp", bufs=6))
        rrp = octx.enter_context(tc.tile_pool(name="rrp", bufs=2))
        rrepp = octx.enter_context(tc.tile_pool(name="rrepp", bufs=2))
        ytp = octx.enter_context(tc.tile_pool(name="ytp", bufs=2))

        # ---------- constants ----------
        tri_sb = const.tile([P, P], BF, tag="tri")
        nc.sync.dma_start(tri_sb, tri_d[:])
        bq_sb = const.tile([P, NT], FP, tag="bq")
        nc.sync.dma_start(bq_sb, bqd[:])
        bk_sb = const.tile([P, NT], FP, tag="bk")
        nc.sync.dma_start(bk_sb, bkd[:])
        bv_sb = const.tile([P, H * DV], BF, tag="bv")
        nc.sync.dma_start(bv_sb, bvd[:])
        bo_sb = const.tile([1, C], BF, tag="bo")
        nc.sync.dma_start(bo_sb, bod[:])
        borep_sb = const.tile([P, C], BF, tag="borep")
        nc.gpsimd.partition_broadcast(borep_sb, bo_sb)
        # warm the ACT exp table during the DMA lead-in
        warm_sb = const.tile([1, P], FP, tag="warm")
        nc.scalar.activation(warm_sb, tri_sb[0:1, :], AFT.Exp)

        # ---------- big resident tensors ----------
        qT_sb = big.tile([P, NT, S], BF, tag="qT")
        kT2_sb = big.tile([P, H, S], BF, tag="kT2")
        v_sb = big.tile([P, NT, H, DV + 1], BF, tag="v")
        oT_sb = big.tile([P, NT, S], BF, tag="oT")
        wo_sb = big.tile([P, NT, C], BF, tag="wo")
        xq_sb = big.tile([P, NT, S], BF, tag="xq")
        xk_sb = big.tile([P, NT, S], BF, tag="xk")
        wv_sb = big.tile([P, NT, H * DV], BF, tag="wv")

        # ones column 64 -> AV matmul row 64 is the softmax denominator
        nc.vector.memset(v_sb[:, :, :, DV], 1.0)
        # zero the unused half of each head's K^T tile (zero lhsT rows nullify
        # the other head's Q rows in the packed 128-contraction)
        for h in range(H):
            hz = DK if h % 2 == 0 else 0
            nc.gpsimd.memset(kT2_sb[hz : hz + DK, h, :], 0.0)

        # ---------- streamed-DMA tiles ----------
        wk_tiles: dict = {}
        wq_tiles: dict = {}
        xv_tiles: dict = {}

        def dma_wk(m, pool, tag="w"):
            t = pool.tile([P, NT, P], BF, tag=tag, name=f"wk{m}")
            nc.sync.dma_start(t, wk[m])
            wk_tiles[m] = t

        def dma_wq(m, pool, tag="w"):
            t = pool.tile([P, NT, P], BF, tag=tag, name=f"wq{m}")
            nc.sync.dma_start(t, wq[m])
            wq_tiles[m] = t

        def dma_xv(m, pool, tag="w"):
            t = pool.tile([P, NT, P], BF, tag=tag, name=f"xv{m}")
            nc.sync.dma_start(t, xv[m])
            xv_tiles[m] = t

        def dma_xhalf(x_sb, x_d, n):
            for kc in range(NT):
                nc.sync.dma_start(
                    x_sb[:, kc, n * CH : (n + 1) * CH], x_d[n][:, kc]
                )

        def dma_wvhalf(n):
            for kc in range(NT):
                nc.sync.dma_start(
                    wv_sb[:, kc, n * CH : (n + 1) * CH], wv[n][:, kc]
                )

        def dma_wo(kc_lo, kc_hi):
            for kc in range(kc_lo, kc_hi):
                nc.sync.dma_start(wo_sb[:, kc], wo_r[:, kc])

        # ---------- projection / output matmul groups (~1.7us PE each) ----------
        def k_group(m, n):
            w = wk_tiles[m]
            py = ppy.tile([P, CH], FP, tag="py", name=f"pk{m}{n}")
            for kc in range(NT):
                nc.tensor.matmul(
                    py,
                    w[:, kc, :],
                    xk_sb[:, kc, n * CH : (n + 1) * CH],
                    start=(kc == 0),
                    stop=(kc == NT - 1),
                )
            sl = slice(n * CH, (n + 1) * CH)
            nc.vector.tensor_scalar_add(
                kT2_sb[0:DK, 2 * m, sl], py[0:DK], bk_sb[0:DK, m : m + 1]
            )
            nc.vector.tensor_scalar_add(
                kT2_sb[DK:P, 2 * m + 1, sl], py[DK:P], bk_sb[DK:P, m : m + 1]
            )

        def q_group(m, n):
            w = wq_tiles[m]
            py = ppy.tile([P, CH], FP, tag="py", name=f"pq{m}{n}")
            for kc in range(NT):
                nc.tensor.matmul(
                    py,
                    w[:, kc, :],
                    xq_sb[:, kc, n * CH : (n + 1) * CH],
                    start=(kc == 0),
                    stop=(kc == NT - 1),
                )
            nc.vector.tensor_scalar_add(
                qT_sb[:, m, n * CH : (n + 1) * CH], py, bq_sb[:, m : m + 1]
            )

        def v_group(m, n):
            xvt = xv_tiles[m]
            py = ppy.tile([P, CH], FP, tag="py", name=f"pv{m}{n}")
            for kc in range(NT):
                nc.tensor.matmul(
                    py,
                    xvt[:, kc, :],
                    wv_sb[:, kc, n * CH : (n + 1) * CH],
                    start=(kc == 0),
                    stop=(kc == NT - 1),
                )
            dst = v_sb[:, m, 8 * n : 8 * (n + 1), 0:DV]
            nc.vector.tensor_tensor(
                dst,
                py.rearrange("p (h v) -> p h v", v=DV),
                bv_sb[:, n * CH : (n + 1) * CH].rearrange("p (h v) -> p h v", v=DV),
                ALU.add,
            )

        def o_group(m, n):
            py = ppy.tile([P, CH], FP, tag="py", name=f"py{m}{n}")
            for kc in range(NT):
                nc.tensor.matmul(
                    py,
                    oT_sb[:, kc, m * P : (m + 1) * P],
                    wo_sb[:, kc, n * CH : (n + 1) * CH],
                    start=(kc == 0),
                    stop=(kc == NT - 1),
                )
            yt = ytp.tile([P, CH], FP, tag="y", name=f"yt{m}{n}")
            nc.vector.tensor_tensor(
                yt, py, borep_sb[:, n * CH : (n + 1) * CH], ALU.add
            )
            nc.sync.dma_start(y_r[:, m, n * CH : (n + 1) * CH], yt)

        # ---------- work queue ----------
        # All projection/output groups flow through one FIFO; attention rounds
        # pull items as PE fillers while the ACT engine chews on exp.
        queue: list = []
        qpos = [0]

        def pull(fence=None):
            limit = len(queue) if fence is None else fence
            if qpos[0] < limit:
                queue[qpos[0]]()
                qpos[0] += 1
                return True
            return False

        def drain(upto):
            while qpos[0] < upto:
                pull()

        # ---------- attention round for one head pair / q-chunk ----------
        def attn(hp, jc, max_fill, fence=None):
            i_list = list(range(4 * jc + 4))
            n_i = len(i_list)
            lag = 3  # AV trails St by `lag` i-steps to hide exp latency
            pos = {}
            pchs = {s: {} for s in (0, 1)}
            offs = {}
            fills = [0]

            def fill():
                # never pull past `fence`: items beyond the next unit's marker
                # may depend on THIS (or a later) attention round's oT writes
                if fills[0] < max_fill and pull(fence):
                    fills[0] += 1

            def st_step(i):
                off = max(0, i * P - jc * CH)
                w = CH - off
                offs[i] = off
                for sub in (0, 1):
                    h = 2 * hp + sub
                    stt = pst.tile(
                        [P, CH], FP, tag="st", name=f"st{hp}_{jc}_{i}_{sub}"
                    )[:, :w]
                    nc.tensor.matmul(
                        stt,
                        kT2_sb[:, h, i * P : (i + 1) * P],
                        qT_sb[:, hp, jc * CH + off : (jc + 1) * CH],
                        start=True,
                        stop=True,
                    )
                    pch = pchp.tile(
                        [P, CH], BF, tag="p", name=f"p{hp}_{jc}_{i}_{sub}"
                    )[:, :w]
                    nc.scalar.activation(pch, stt, AFT.Exp)
                    if i * P >= jc * CH:
                        nc.vector.tensor_tensor(
                            pch[:, 0:P], pch[:, 0:P], tri_sb, ALU.mult
                        )
                    pchs[sub][i] = pch

            def av_step(i):
                off = offs[i]
                for sub in (0, 1):
                    h = 2 * hp + sub
                    if i == 0:
                        pos[sub] = ppo.tile(
                            [P, CH], FP, tag="o", name=f"o{hp}_{jc}_{sub}"
                        )[: DV + 1]
                    nc.tensor.matmul(
                        pos[sub][:, off:],
                        v_sb[:, i, h, :],
                        pchs[sub][i],
                        start=(i == 0),
                        stop=(i == n_i - 1),
                    )

            for i in i_list[:lag]:
                st_step(i)
                if i >= 1:
                    fill()
            for i in i_list[lag:]:
                st_step(i)
                fill()
                av_step(i - lag)
            for i in i_list[n_i - lag :]:
                av_step(i)
                fill()

            for sub in (0, 1):
                po = pos[sub]
                # reciprocal_approx_fast only works at partition base 0, so
                # stage the denominator row 64 -> 0 with a stock copy first
                rc = rrp.tile([1, CH], FP, tag="rc", name=f"rc{hp}_{jc}_{sub}")
                nc.vector.tensor_copy(out=rc, in_=po[DV : DV + 1, :])
                rr = rrp.tile([1, CH], FP, tag="rr", name=f"rr{hp}_{jc}_{sub}")
                nc.vector.reciprocal_approx_fast(rr, rc)
                rrep = rrepp.tile(
                    [DV, CH], FP, tag="rrep", name=f"rrep{hp}_{jc}_{sub}"
                )
                nc.gpsimd.partition_broadcast(rrep, rr)
                nc.vector.tensor_tensor(
                    oT_sb[sub * DV : (sub + 1) * DV, hp, jc * CH : (jc + 1) * CH],
                    po[0:DV],
                    rrep,
                    ALU.mult,
                )

        # ================= schedule =================
        # lead-in DMAs (consts already queued above)
        dma_wk(0, const, "wk0")
        dma_xhalf(xk_sb, xk, 0)
        dma_wq(0, const, "wq0")
        dma_xhalf(xq_sb, xq, 0)
        for m in range(4):
            dma_xv(m, const, f"xv{m}")
        dma_wvhalf(0)
        dma_wk(1, wkp)
        dma_xhalf(xk_sb, xk, 1)

        def KG(m, n):
            return lambda: k_group(m, n)

        def QG(m, n):
            return lambda: q_group(m, n)

        def VG(m, n):
            return lambda: v_group(m, n)

        def OG(m, n):
            return lambda: o_group(m, n)

        # units: (attn args, groups due before it, dma prefetch at drain time)
        units = [
            ((0, 0, 2), [KG(0, 0), QG(0, 0), VG(0, 0), VG(1, 0), VG(2, 0), VG(3, 0)],
             lambda: (dma_wq(1, wqp), dma_wvhalf(1))),
            ((1, 0, 2), [KG(1, 0), KG(1, 1), QG(1, 0), VG(0, 1)],
             lambda: (dma_wk(2, wkp), dma_wq(2, wqp))),
            ((2, 0, 2), [KG(2, 0), KG(2, 1), QG(2, 0), VG(1, 1)],
             lambda: (dma_wk(3, wkp), dma_wq(3, wqp))),
            ((3, 0, 2), [KG(3, 0), KG(3, 1), QG(3, 0), VG(2, 1)],
             lambda: (dma_wk(4, wkp), dma_wq(4, wqp))),
            ((4, 0, 2), [VG(3, 1), KG(4, 0), KG(4, 1), QG(4, 0)],
             lambda: (dma_wk(5, wkp), dma_wq(5, wqp), dma_xv(4, xvp))),
            ((5, 0, 2), [KG(5, 0), KG(5, 1), QG(5, 0), VG(4, 0), VG(4, 1)],
             lambda: (dma_wk(6, wkp), dma_wq(6, wqp), dma_xv(5, xvp))),
            ((6, 0, 2), [KG(6, 0), KG(6, 1), QG(6, 0), VG(5, 0), VG(5, 1)],
             lambda: (dma_wk(7, wkp), dma_wq(7, wqp), dma_xv(6, xvp),
                      dma_wq(1, wqp))),
            ((7, 0, 2), [KG(7, 0), KG(7, 1), QG(7, 0), VG(6, 0), VG(6, 1)],
             lambda: (dma_xv(7, xvp), dma_xhalf(xq_sb, xq, 1),
                      dma_wq(2, wqp), dma_wo(0, 4))),
            ((0, 1, 3), [KG(0, 1), VG(7, 0), VG(7, 1), QG(0, 1), QG(1, 1)],
             lambda: (dma_wq(3, wqp), dma_wo(4, 8))),
            ((1, 1, 3), [QG(2, 1), OG(0, 0)],
             lambda: dma_wq(4, wqp)),
            ((2, 1, 3), [QG(3, 1), OG(0, 1)],
             lambda: dma_wq(5, wqp)),
            ((3, 1, 3), [QG(4, 1), OG(1, 0)],
             lambda: dma_wq(6, wqp)),
            ((4, 1, 3), [QG(5, 1), OG(1, 1)],
             lambda: dma_wq(7, wqp)),
            ((5, 1, 3), [QG(6, 1), OG(2, 0)], None),
            ((6, 1, 3), [QG(7, 1), OG(2, 1)], None),
            ((7, 1, 3), [OG(3, 0), OG(3, 1)], None),
        ]
        tail = []
        for m in range(4, NT):
            tail.append(OG(m, 0))
            tail.append(OG(m, 1))

        markers = []
        for _, groups, _ in units:
            queue.extend(groups)
            markers.append(len(queue))
        queue.extend(tail)

        fences = markers[1:] + [markers[-1]]
        for u, (args, _, dma_fn) in enumerate(units):
            if dma_fn is not None:
                dma_fn()
            drain(markers[u])
            attn(*args, fence=fences[u])
        drain(len(queue))

    nc.finalize()
    return nc


_NC_CACHE = None


def _get_nc() -> bass.Bass:
    global _NC_CACHE
    if _NC_CACHE is None:
        _NC_CACHE = build_nc()
    return _NC_CACHE


def prep_shared(Wq, bq, Wk, bk, Wv, bv, Wo, bo):
    """Host-side packing of weights/biases (shared by all cores)."""
    scale = 1.0 / math.sqrt(DK)
    Wq = np.asarray(Wq, np.float32)
    Wk = np.asarray(Wk, np.float32)
    Wv = np.asarray(Wv, np.float32)
    Wo = np.asarray(Wo, np.float32)
    WqT = (Wq.transpose(1, 0, 2).reshape(C, H * DK) * scale).astype(BF_NP)
    WkT = Wk.transpose(1, 0, 2).reshape(C, H * DK).astype(BF_NP)
    WvT = Wv.transpose(1, 0, 2).reshape(C, H * DV).astype(BF_NP)
    out = {
        # [m, p, kc, mi] <- W[kc*128+p, m*128+mi]
        "wq": np.ascontiguousarray(
            WqT.reshape(NT, P, NT, P).transpose(2, 1, 0, 3)
        ),
        "wk": np.ascontiguousarray(
            WkT.reshape(NT, P, NT, P).transpose(2, 1, 0, 3)
        ),
        # [n, p, kc, j] <- W[kc*128+p, n*512+j]
        "wv": np.ascontiguousarray(
            WvT.reshape(NT, P, NCH, CH).transpose(2, 1, 0, 3)
        ),
        "wo": Wo.astype(BF_NP),
        "bq": np.ascontiguousarray(
            (np.asarray(bq, np.float32).reshape(H * DK) * scale)
            .reshape(NT, P)
            .T.astype(np.float32)
        ),
        "bk": np.ascontiguousarray(
            np.asarray(bk, np.float32).reshape(NT, P).T.astype(np.float32)
        ),
        "bv": np.ascontiguousarray(
            np.broadcast_to(
                np.asarray(bv, np.float32).reshape(1, H * DV), (P, H * DV)
            ).astype(BF_NP)
        ),
        "bo": np.ascontiguousarray(
            np.asarray(bo, np.float32).reshape(1, C).astype(BF_NP)
        ),
    }
    return out


def prep_core(q_embs_b, k_embs_b, v_embs_b):
    XqT = np.asarray(q_embs_b, np.float32).T.astype(BF_NP)  # [C, S]
    XkT = np.asarray(k_embs_b, np.float32).T.astype(BF_NP)
    XvT = np.asarray(v_embs_b, np.float32).T.astype(BF_NP)
    return {
        # [n, p, kc, j] <- X^T[kc*128+p, n*512+j]
        "xq": np.ascontiguousarray(
            XqT.reshape(NT, P, NCH, CH).transpose(2, 1, 0, 3)
        ),
        "xk": np.ascontiguousarray(
            XkT.reshape(NT, P, NCH, CH).transpose(2, 1, 0, 3)
        ),
        # [m, p, kc, mi] <- X^T[kc*128+p, m*128+mi]
        "xv": np.ascontiguousarray(
            XvT.reshape(NT, P, NT, P).transpose(2, 1, 0, 3)
        ),
    }


def kernel(q_embs, k_embs, v_embs, Wq, bq, Wk, bk, Wv, bv, Wo, bo, **run_kwargs):
    nc = _get_nc()
    shared = prep_shared(Wq, bq, Wk, bk, Wv, bv, Wo, bo)
    q_embs = np.asarray(q_embs, np.float32)
    k_embs = np.asarray(k_embs, np.float32)
    v_embs = np.asarray(v_embs, np.float32)
    in_maps = []
    for b in range(B):
        m = dict(shared)
        m.update(prep_core(q_embs[b], k_embs[b], v_embs[b]))
        in_maps.append(m)
    res = run_bass_kernel_spmd(nc, in_maps, core_ids=list(range(B)), **run_kwargs)
    out = np.stack([res.results[i]["y"] for i in range(B)], axis=0)
    if run_kwargs:
        kernel.last_results = res
    return out


if __name__ == "__main__":
    rng = np.random.default_rng(0)
    inputs = {
        "q_embs": rng.standard_normal((B, S, C), np.float32),
        "k_embs": rng.standard_normal((B, S, C), np.float32),
        "v_embs": rng.standard_normal((B, S, C), np.float32),
        "Wq": rng.standard_normal((H, C, DK), np.float32) * 0.02,
        "bq": np.zeros((H, DK), np.float32),
        "Wk": rng.standard_normal((H, C, DK), np.float32) * 0.02,
        "bk": np.zeros((H, DK), np.float32),
        "Wv": rng.standard_normal((H, C, DV), np.float32) * 0.02,
        "bv": np.zeros((H, DV), np.float32),
        "Wo": rng.standard_normal((H * DV, C), np.float32) * 0.02,
        "bo": np.zeros((C,), np.float32),
    }
    out = kernel(**inputs)
    print(out.shape, out.dtype)


# revision 18
# speedup vs baseline: 1.3310x; 1.2002x over previous
"""Multi-head causal attention (B=8, S=1024, C=1024, H=16, dk=dv=64) on 8 trn2 cores.

Sharding: data-parallel over batch. Each NeuronCore processes one batch element
end-to-end (projections + attention + output projection); no collectives.

v2: software-pipelined single-pass schedule. The K/Q/V projection matmul
groups are streamed *into* the attention phase (which is ACT/exp-bound) so the
PE never idles; weights arrive via just-in-time DMA of host-repacked
contiguous slices. The softmax denominator reciprocal moved from the ACT
(Ln/Exp chain) to a single DVE reciprocal_approx_fast reading PSUM directly.

Per-core math (all bf16 matmuls, fp32 PSUM):
  QT = wq.T @ xq  -> [H*DK, S]   (head-pair-major rows, pre-scaled 1/sqrt(dk))
  KT = wk.T @ xk  -> per-head [64, S] tiles zero-padded to 128 partitions
  V  = xv.T @ wv  -> [S, H*DV]   (+ ones column per head for the denominator)
  per (head pair, q-chunk): St[t,q] -> exp -> tri-mask -> O^T (+ denom row)
    via matmul(lhsT=[V_h | 1], rhs=P); normalize with DVE 1/r * broadcast
  Y = oT.T-contract @ wo + bo -> [S, C] f32
"""

import math
import os
import sys

import numpy as np

try:
    import concourse.bass as bass
except ImportError:  # make concourse importable in a bare grading dir
    for _p in ("/opt/trn_rl_repo", os.path.expanduser("~/.axon_site/_ro/trn_rl_repo")):
        if os.path.isdir(_p) and _p not in sys.path:
            sys.path.insert(0, _p)
    import concourse.bass as bass

from contextlib import ExitStack

import ml_dtypes

import concourse.mybir as mybir
import concourse.tile as tile
from concourse import bacc
from concourse.bass_utils import run_bass_kernel_spmd


def _setup_act_tables():
    """Pin the ACT function table to the set that covers exp+ln+identity+copy
    so the kernel never reloads LUTs mid-flight."""
    import json
    import shutil
    import tempfile

    import concourse.hw_specs as hw_specs
    from concourse import bacc as _bacc

    if os.environ.get("BASS_ACT_ROOT_JSON_PATH"):
        return  # already configured
    from neuronxcc.driver.Job import Job

    orig = os.path.join(
        Job.getPackageDir(), "pwp", "pwp_bin_trainium", "act_info.json"
    )
    assert os.path.isfile(orig), orig
    dst = os.path.join(tempfile.gettempdir(), "mha_act_tables")
    if not os.path.isdir(dst):
        tmp = dst + ".tmp"
        shutil.rmtree(tmp, ignore_errors=True)
        shutil.copytree(os.path.dirname(orig), tmp)
        with open(os.path.join(tmp, "act_info.json")) as f:
            info = json.load(f)
        sets = info["act_func_sets"]
        want = [s for s in sets if s["name"] == "natural_log_exp_and_others"]
        rest = [s for s in sets if s["name"] != "natural_log_exp_and_others"]
        info["act_func_sets"] = want + rest
        with open(os.path.join(tmp, "act_info.json"), "w") as f:
            json.dump(info, f)
        os.replace(tmp, dst)
    path = os.path.join(dst, "act_info.json")
    os.environ["BASS_ACT_ROOT_JSON_PATH"] = path

    def patched(module_arch):
        with open(path) as af:
            act_info = json.load(af)
        return {
            ent["name"]: {
                mybir.ActivationFunctionType.from_pwp(v) for v in ent["act"].keys()
            }
            for ent in act_info["act_func_sets"]
        }

    hw_specs.get_activation_tables = patched
    _bacc.get_activation_tables = patched
    from concourse import bass_interp as _bi

    _bi.get_activation_tables = patched


B, S, C = 8, 1024, 1024
H, DK, DV = 16, 64, 64
P = 128
NT = 8  # number of 128-tiles along S / C / H*DK
CH = 512  # free-dim chunk (one PSUM bank of fp32)
NCH = S // CH
NPAIR = H // 2

FP = mybir.dt.float32
BF = mybir.dt.bfloat16
BF_NP = ml_dtypes.bfloat16
AFT = mybir.ActivationFunctionType
ALU = mybir.AluOpType


def build_nc() -> bass.Bass:
    _setup_act_tables()
    nc = bacc.Bacc()

    # host-packed inputs; see prep_shared/prep_core for layouts
    xq = nc.dram_tensor("xq", [NCH, P, NT, CH], BF, kind="ExternalInput")
    xk = nc.dram_tensor("xk", [NCH, P, NT, CH], BF, kind="ExternalInput")
    xv = nc.dram_tensor("xv", [NT, P, NT, P], BF, kind="ExternalInput")
    wq = nc.dram_tensor("wq", [NT, P, NT, P], BF, kind="ExternalInput")
    wk = nc.dram_tensor("wk", [NT, P, NT, P], BF, kind="ExternalInput")
    wv = nc.dram_tensor("wv", [NCH, P, NT, CH], BF, kind="ExternalInput")
    wo = nc.dram_tensor("wo", [H * DV, C], BF, kind="ExternalInput")
    bqd = nc.dram_tensor("bq", [P, NT], FP, kind="ExternalInput")
    bkd = nc.dram_tensor("bk", [P, NT], FP, kind="ExternalInput")
    bvd = nc.dram_tensor("bv", [P, H * DV], BF, kind="ExternalInput")
    bod = nc.dram_tensor("bo", [1, C], BF, kind="ExternalInput")
    y = nc.dram_tensor("y", [S, C], FP, kind="ExternalOutput")

    # binary causal mask [t,q] for the 128-wide diagonal block (1 iff t<=q)
    tri_d = nc.inline_tensor(
        np.triu(np.ones((P, P), np.float32)).astype(BF_NP), "tri"
    )

    wo_r = wo.rearrange("(ko p) c -> p ko c", p=P)
    y_r = y.rearrange("(mo p) c -> p mo c", p=P)

    with tile.TileContext(nc) as tc, ExitStack() as octx:
        const = octx.enter_context(tc.tile_pool(name="const", bufs=1))
        big = octx.enter_context(tc.tile_pool(name="big", bufs=1))
        wkp = octx.enter_context(tc.tile_pool(name="wkp", bufs=3))
        wqp = octx.enter_context(tc.tile_pool(name="wqp", bufs=3))
        xvp = octx.enter_context(tc.tile_pool(name="xvp", bufs=3))
        pst = octx.enter_context(tc.tile_pool(name="pst", bufs=4, space="PSUM"))
        ppo = octx.enter_context(tc.tile_pool(name="ppo", bufs=2, space="PSUM"))
        ppy = octx.enter_context(tc.tile_pool(name="ppy", bufs=2, space="PSUM"))
        pchp = octx.enter_context(tc.tile_pool(name="pchp", bufs=6))
        rrp = octx.enter_context(tc.tile_pool(name="rrp", bufs=2))
        rrepp = octx.enter_context(tc.tile_pool(name="rrepp", bufs=2))
        ytp = octx.enter_context(tc.tile_pool(name="ytp", bufs=2))

        # ---------- constants ----------
        tri_sb = const.tile([P, P], BF, tag="tri")
        nc.sync.dma_start(tri_sb, tri_d[:])
        bq_sb = const.tile([P, NT], FP, tag="bq")
        nc.sync.dma_start(bq_sb, bqd[:])
        bk_sb = const.tile([P, NT], FP, tag="bk")
        nc.sync.dma_start(bk_sb, bkd[:])
        bv_sb = const.tile([P, H * DV], BF, tag="bv")
        nc.sync.dma_start(bv_sb, bvd[:])
        bo_sb = const.tile([1, C], BF, tag="bo")
        nc.sync.dma_start(bo_sb, bod[:])
        borep_sb = const.tile([P, C], BF, tag="borep")
        nc.gpsimd.partition_broadcast(borep_sb, bo_sb)
        # warm the ACT exp table during the DMA lead-in
        warm_sb = const.tile([1, P], FP, tag="warm")
        nc.scalar.activation(warm_sb, tri_sb[0:1, :], AFT.Exp)

        # ---------- big resident tensors ----------
        qT_sb = big.tile([P, NT, S], BF, tag="qT")
        kT2_sb = big.tile([P, H, S], BF, tag="kT2")
        v_sb = big.tile([P, NT, H, DV + 1], BF, tag="v")
        oT_sb = big.tile([P, NT, S], BF, tag="oT")
        wo_sb = big.tile([P, NT, C], BF, tag="wo")
        xq_sb = big.tile([P, NT, S], BF, tag="xq")
        xk_sb = big.tile([P, NT, S], BF, tag="xk")
        wv_sb = big.tile([P, NT, H * DV], BF, tag="wv")

        # ones column 64 -> AV matmul row 64 is the softmax denominator
        nc.vector.memset(v_sb[:, :, :, DV], 1.0)
        # zero the unused half of each head's K^T tile (zero lhsT rows nullify
        # the other head's Q rows in the packed 128-contraction)
        for h in range(H):
            hz = DK if h % 2 == 0 else 0
            nc.gpsimd.memset(kT2_sb[hz : hz + DK, h, :], 0.0)

        # ---------- streamed-DMA tiles ----------
        wk_tiles: dict = {}
        wq_tiles: dict = {}
        xv_tiles: dict = {}

        def dma_wk(m, pool, tag="w"):
            t = pool.tile([P, NT, P], BF, tag=tag, name=f"wk{m}")
            nc.sync.dma_start(t, wk[m])
            wk_tiles[m] = t

        def dma_wq(m, pool, tag="w"):
            t = pool.tile([P, NT, P], BF, tag=tag, name=f"wq{m}")
            nc.sync.dma_start(t, wq[m])
            wq_tiles[m] = t

        def dma_xv(m, pool, tag="w"):
            t = pool.tile([P, NT, P], BF, tag=tag, name=f"xv{m}")
            nc.sync.dma_start(t, xv[m])
            xv_tiles[m] = t

        def dma_xhalf(x_sb, x_d, n):
            for h in (0, 1):
                nc.sync.dma_start(
                    x_sb[:, 4 * h : 4 * h + 4, n * CH : (n + 1) * CH],
                    x_d[n][:, 4 * h : 4 * h + 4],
                )

        def dma_wvhalf(n):
            nc.sync.dma_start(wv_sb[:, :, n * CH : (n + 1) * CH], wv[n])

        def dma_wo(kc_lo, kc_hi):
            nc.sync.dma_start(wo_sb[:, kc_lo:kc_hi], wo_r[:, kc_lo:kc_hi])

        # ---------- projection / output matmul groups (~1.7us PE each) ----------
        def k_group(m, n):
            w = wk_tiles[m]
            py = ppy.tile([P, CH], FP, tag="py", name=f"pk{m}{n}")
            for kc in range(NT):
                nc.tensor.matmul(
                    py,
                    w[:, kc, :],
                    xk_sb[:, kc, n * CH : (n + 1) * CH],
                    start=(kc == 0),
                    stop=(kc == NT - 1),
                )
            sl = slice(n * CH, (n + 1) * CH)
            nc.scalar.activation(
                kT2_sb[0:DK, 2 * m, sl], py[0:DK], AFT.Identity,
                bias=bk_sb[0:DK, m : m + 1],
            )
            nc.scalar.activation(
                kT2_sb[DK:P, 2 * m + 1, sl], py[DK:P], AFT.Identity,
                bias=bk_sb[DK:P, m : m + 1],
            )

        def q_group(m, n):
            w = wq_tiles[m]
            py = ppy.tile([P, CH], FP, tag="py", name=f"pq{m}{n}")
            for kc in range(NT):
                nc.tensor.matmul(
                    py,
                    w[:, kc, :],
                    xq_sb[:, kc, n * CH : (n + 1) * CH],
                    start=(kc == 0),
                    stop=(kc == NT - 1),
                )
            nc.scalar.activation(
                qT_sb[:, m, n * CH : (n + 1) * CH], py, AFT.Identity,
                bias=bq_sb[:, m : m + 1],
            )

        def v_group(m, n):
            xvt = xv_tiles[m]
            py = ppy.tile([P, CH], FP, tag="py", name=f"pv{m}{n}")
            for kc in range(NT):
                nc.tensor.matmul(
                    py,
                    xvt[:, kc, :],
                    wv_sb[:, kc, n * CH : (n + 1) * CH],
                    start=(kc == 0),
                    stop=(kc == NT - 1),
                )
            dst = v_sb[:, m, 8 * n : 8 * (n + 1), 0:DV]
            nc.vector.tensor_tensor(
                dst,
                py.rearrange("p (h v) -> p h v", v=DV),
                bv_sb[:, n * CH : (n + 1) * CH].rearrange("p (h v) -> p h v", v=DV),
                ALU.add,
            )

        def o_group(m, n):
            py = ppy.tile([P, CH], FP, tag="py", name=f"py{m}{n}")
            for kc in range(NT):
                nc.tensor.matmul(
                    py,
                    oT_sb[:, kc, m * P : (m + 1) * P],
                    wo_sb[:, kc, n * CH : (n + 1) * CH],
                    start=(kc == 0),
                    stop=(kc == NT - 1),
                )
            yt = ytp.tile([P, CH], FP, tag="y", name=f"yt{m}{n}")
            nc.vector.tensor_tensor(
                yt, py, borep_sb[:, n * CH : (n + 1) * CH], ALU.add
            )
            nc.sync.dma_start(y_r[:, m, n * CH : (n + 1) * CH], yt)

        # ---------- work queue ----------
        # All projection/output groups flow through one FIFO; attention rounds
        # pull items as PE fillers while the ACT engine chews on exp.
        queue: list = []
        qpos = [0]

        def pull(fence=None):
            limit = len(queue) if fence is None else fence
            if qpos[0] < limit:
                queue[qpos[0]]()
                qpos[0] += 1
                return True
            return False

        def drain(upto):
            while qpos[0] < upto:
                pull()

        # ---------- attention round for one head pair / q-chunk ----------
        def attn(hp, jc, max_fill, fence=None):
            i_list = list(range(4 * jc + 4))
            n_i = len(i_list)
            lag = 3  # AV trails St by `lag` i-steps to hide exp latency
            pos = {}
            pchs = {s: {} for s in (0, 1)}
            offs = {}
            fills = [0]

            def fill():
                # never pull past `fence`: items beyond the next unit's marker
                # may depend on THIS (or a later) attention round's oT writes
                if fills[0] < max_fill and pull(fence):
                    fills[0] += 1

            def st_step(i):
                off = max(0, i * P - jc * CH)
                w = CH - off
                offs[i] = off
                for sub in (0, 1):
                    h = 2 * hp + sub
                    stt = pst.tile(
                        [P, CH], FP, tag="st", name=f"st{hp}_{jc}_{i}_{sub}"
                    )[:, :w]
                    nc.tensor.matmul(
                        stt,
                        kT2_sb[:, h, i * P : (i + 1) * P],
                        qT_sb[:, hp, jc * CH + off : (jc + 1) * CH],
                        start=True,
                        stop=True,
                    )
                    pch = pchp.tile(
                        [P, CH], BF, tag="p", name=f"p{hp}_{jc}_{i}_{sub}"
                    )[:, :w]
                    nc.scalar.activation(pch, stt, AFT.Exp)
                    if i * P >= jc * CH:
                        nc.vector.tensor_tensor(
                            pch[:, 0:P], pch[:, 0:P], tri_sb, ALU.mult
                        )
                    pchs[sub][i] = pch

            def av_step(i):
                off = offs[i]
                for sub in (0, 1):
                    h = 2 * hp + sub
                    if i == 0:
                        pos[sub] = ppo.tile(
                            [P, CH], FP, tag="o", name=f"o{hp}_{jc}_{sub}"
                        )[: DV + 1]
                    nc.tensor.matmul(
                        pos[sub][:, off:],
                        v_sb[:, i, h, :],
                        pchs[sub][i],
                        start=(i == 0),
                        stop=(i == n_i - 1),
                    )

            for i in i_list[:lag]:
                st_step(i)
                if i >= 1:
                    fill()
            for i in i_list[lag:]:
                st_step(i)
                fill()
                av_step(i - lag)
            for i in i_list[n_i - lag :]:
                av_step(i)
                fill()

            for sub in (0, 1):
                po = pos[sub]
                # reciprocal_approx_fast only works at partition base 0, so
                # stage the denominator row 64 -> 0 with a stock copy first
                rc = rrp.tile([1, CH], FP, tag="rc", name=f"rc{hp}_{jc}_{sub}")
                nc.scalar.activation(rc, po[DV : DV + 1, :], AFT.Copy)
                rr = rrp.tile([1, CH], FP, tag="rr", name=f"rr{hp}_{jc}_{sub}")
                nc.vector.reciprocal_approx_fast(rr, rc)
                rrep = rrepp.tile(
                    [DV, CH], FP, tag="rrep", name=f"rrep{hp}_{jc}_{sub}"
                )
                nc.gpsimd.partition_broadcast(rrep, rr)
                nc.vector.tensor_tensor(
                    oT_sb[sub * DV : (sub + 1) * DV, hp, jc * CH : (jc + 1) * CH],
                    po[0:DV],
                    rrep,
                    ALU.mult,
                )

        # ================= schedule =================
        # lead-in DMAs (consts already queued above)
        dma_wk(0, const, "wk0")
        dma_xhalf(xk_sb, xk, 0)
        dma_wq(0, const, "wq0")
        dma_xhalf(xq_sb, xq, 0)
        for m in range(4):
            dma_xv(m, const, f"xv{m}")
        dma_wvhalf(0)
        dma_wk(1, wkp)
        dma_xhalf(xk_sb, xk, 1)

        def KG(m, n):
            return lambda: k_group(m, n)

        def QG(m, n):
            return lambda: q_group(m, n)

        def VG(m, n):
            return lambda: v_group(m, n)

        def OG(m, n):
            return lambda: o_group(m, n)

        # units: (attn args, groups due before it, dma prefetch at drain time)
        units = [
            ((0, 0, 2), [KG(0, 0), QG(0, 0), VG(0, 0), VG(1, 0), VG(2, 0), VG(3, 0)],
             lambda: (dma_wq(1, wqp), dma_wvhalf(1))),
            ((1, 0, 2), [KG(1, 0), KG(1, 1), QG(1, 0), VG(0, 1)],
             lambda: (dma_wk(2, wkp), dma_wq(2, wqp))),
            ((2, 0, 2), [KG(2, 0), KG(2, 1), QG(2, 0), VG(1, 1)],
             lambda: (dma_wk(3, wkp), dma_wq(3, wqp))),
            ((3, 0, 2), [KG(3, 0), KG(3, 1), QG(3, 0), VG(2, 1)],
             lambda: (dma_wk(4, wkp), dma_wq(4, wqp))),
            ((4, 0, 2), [VG(3, 1), KG(4, 0), KG(4, 1), QG(4, 0)],
             lambda: (dma_wk(5, wkp), dma_wq(5, wqp), dma_xv(4, xvp))),
            ((5, 0, 2), [KG(5, 0), KG(5, 1), QG(5, 0), VG(4, 0), VG(4, 1)],
             lambda: (dma_wk(6, wkp), dma_wq(6, wqp), dma_xv(5, xvp))),
            ((6, 0, 2), [KG(6, 0), KG(6, 1), QG(6, 0), VG(5, 0), VG(5, 1)],
             lambda: (dma_wk(7, wkp), dma_wq(7, wqp), dma_xv(6, xvp),
                      dma_wq(1, wqp))),
            ((7, 0, 2), [KG(7, 0), KG(7, 1), QG(7, 0), VG(6, 0), VG(6, 1)],
             lambda: (dma_xv(7, xvp), dma_xhalf(xq_sb, xq, 1),
                      dma_wq(2, wqp), dma_wo(0, 4))),
            ((0, 1, 3), [KG(0, 1), VG(7, 0), VG(7, 1), QG(0, 1), QG(1, 1)],
             lambda: (dma_wq(3, wqp), dma_wo(4, 8))),
            ((1, 1, 3), [QG(2, 1), OG(0, 0)],
             lambda: dma_wq(4, wqp)),
            ((2, 1, 3), [QG(3, 1), OG(0, 1)],
             lambda: dma_wq(5, wqp)),
            ((3, 1, 3), [QG(4, 1), OG(1, 0)],
             lambda: dma_wq(6, wqp)),
            ((4, 1, 3), [QG(5, 1), OG(1, 1)],
             lambda: dma_wq(7, wqp)),
            ((5, 1, 3), [QG(6, 1), OG(2, 0)], None),
            ((6, 1, 3), [QG(7, 1), OG(2, 1)], None),
            ((7, 1, 3), [OG(3, 0), OG(3, 1)], None),
        ]
        tail = []
        for m in range(4, NT):
            tail.append(OG(m, 0))
            tail.append(OG(m, 1))

        markers = []
        for _, groups, _ in units:
            queue.extend(groups)
            markers.append(len(queue))
        queue.extend(tail)

        fences = markers[1:] + [markers[-1]]
        for u, (args, _, dma_fn) in enumerate(units):
            if dma_fn is not None:
                dma_fn()
            drain(markers[u])
            attn(*args, fence=fences[u])
        drain(len(queue))

    nc.finalize()
    return nc


_NC_CACHE = None


def _get_nc() -> bass.Bass:
    global _NC_CACHE
    if _NC_CACHE is None:
        _NC_CACHE = build_nc()
    return _NC_CACHE


def prep_shared(Wq, bq, Wk, bk, Wv, bv, Wo, bo):
    """Host-side packing of weights/biases (shared by all cores)."""
    scale = 1.0 / math.sqrt(DK)
    Wq = np.asarray(Wq, np.float32)
    Wk = np.asarray(Wk, np.float32)
    Wv = np.asarray(Wv, np.float32)
    Wo = np.asarray(Wo, np.float32)
    WqT = (Wq.transpose(1, 0, 2).reshape(C, H * DK) * scale).astype(BF_NP)
    WkT = Wk.transpose(1, 0, 2).reshape(C, H * DK).astype(BF_NP)
    WvT = Wv.transpose(1, 0, 2).reshape(C, H * DV).astype(BF_NP)
    out = {
        # [m, p, kc, mi] <- W[kc*128+p, m*128+mi]
        "wq": np.ascontiguousarray(
            WqT.reshape(NT, P, NT, P).transpose(2, 1, 0, 3)
        ),
        "wk": np.ascontiguousarray(
            WkT.reshape(NT, P, NT, P).transpose(2, 1, 0, 3)
        ),
        # [n, p, kc, j] <- W[kc*128+p, n*512+j]
        "wv": np.ascontiguousarray(
            WvT.reshape(NT, P, NCH, CH).transpose(2, 1, 0, 3)
        ),
        "wo": Wo.astype(BF_NP),
        "bq": np.ascontiguousarray(
            (np.asarray(bq, np.float32).reshape(H * DK) * scale)
            .reshape(NT, P)
            .T.astype(np.float32)
        ),
        "bk": np.ascontiguousarray(
            np.asarray(bk, np.float32).reshape(NT, P).T.astype(np.float32)
        ),
        "bv": np.ascontiguousarray(
            np.broadcast_to(
                np.asarray(bv, np.float32).reshape(1, H * DV), (P, H * DV)
            ).astype(BF_NP)
        ),
        "bo": np.ascontiguousarray(
            np.asarray(bo, np.float32).reshape(1, C).astype(BF_NP)
        ),
    }
    return out


def prep_core(q_embs_b, k_embs_b, v_embs_b):
    XqT = np.asarray(q_embs_b, np.float32).T.astype(BF_NP)  # [C, S]
    XkT = np.asarray(k_embs_b, np.float32).T.astype(BF_NP)
    XvT = np.asarray(v_embs_b, np.float32).T.astype(BF_NP)
    return {
        # [n, p, kc, j] <- X^T[kc*128+p, n*512+j]
        "xq": np.ascontiguousarray(
            XqT.reshape(NT, P, NCH, CH).transpose(2, 1, 0, 3)
        ),
        "xk": np.ascontiguousarray(
            XkT.reshape(NT, P, NCH, CH).transpose(2, 1, 0, 3)
        ),
        # [m, p, kc, mi] <- X^T[kc*128+p, m*128+mi]
        "xv": np.ascontiguousarray(
            XvT.reshape(NT, P, NT, P).transpose(2, 1, 0, 3)
        ),
    }


def kernel(q_embs, k_embs, v_embs, Wq, bq, Wk, bk, Wv, bv, Wo, bo, **run_kwargs):
    nc = _get_nc()
    shared = prep_shared(Wq, bq, Wk, bk, Wv, bv, Wo, bo)
    q_embs = np.asarray(q_embs, np.float32)
    k_embs = np.asarray(k_embs, np.float32)
    v_embs = np.asarray(v_embs, np.float32)
    in_maps = []
    for b in range(B):
        m = dict(shared)
        m.update(prep_core(q_embs[b], k_embs[b], v_embs[b]))
        in_maps.append(m)
    res = run_bass_kernel_spmd(nc, in_maps, core_ids=list(range(B)), **run_kwargs)
    out = np.stack([res.results[i]["y"] for i in range(B)], axis=0)
    if run_kwargs:
        kernel.last_results = res
    return out


if __name__ == "__main__":
    rng = np.random.default_rng(0)
    inputs = {
        "q_embs": rng.standard_normal((B, S, C), np.float32),
        "k_embs": rng.standard_normal((B, S, C), np.float32),
        "v_embs": rng.standard_normal((B, S, C), np.float32),
        "Wq": rng.standard_normal((H, C, DK), np.float32) * 0.02,
        "bq": np.zeros((H, DK), np.float32),
        "Wk": rng.standard_normal((H, C, DK), np.float32) * 0.02,
        "bk": np.zeros((H, DK), np.float32),
        "Wv": rng.standard_normal((H, C, DV), np.float32) * 0.02,
        "bv": np.zeros((H, DV), np.float32),
        "Wo": rng.standard_normal((H * DV, C), np.float32) * 0.02,
        "bo": np.zeros((C,), np.float32),
    }
    out = kernel(**inputs)
    print(out.shape, out.dtype)


# revision 21
# speedup vs baseline: 1.3323x; 1.0010x over previous
"""Multi-head causal attention (B=8, S=1024, C=1024, H=16, dk=dv=64) on 8 trn2 cores.

Sharding: data-parallel over batch. Each NeuronCore processes one batch element
end-to-end (projections + attention + output projection); no collectives.

v2: software-pipelined single-pass schedule. The K/Q/V projection matmul
groups are streamed *into* the attention phase (which is ACT/exp-bound) so the
PE never idles; weights arrive via just-in-time DMA of host-repacked
contiguous slices. The softmax denominator reciprocal moved from the ACT
(Ln/Exp chain) to a single DVE reciprocal_approx_fast reading PSUM directly.

Per-core math (all bf16 matmuls, fp32 PSUM):
  QT = wq.T @ xq  -> [H*DK, S]   (head-pair-major rows, pre-scaled 1/sqrt(dk))
  KT = wk.T @ xk  -> per-head [64, S] tiles zero-padded to 128 partitions
  V  = xv.T @ wv  -> [S, H*DV]   (+ ones column per head for the denominator)
  per (head pair, q-chunk): St[t,q] -> exp -> tri-mask -> O^T (+ denom row)
    via matmul(lhsT=[V_h | 1], rhs=P); normalize with DVE 1/r * broadcast
  Y = oT.T-contract @ wo + bo -> [S, C] f32
"""

import math
import os
import sys

import numpy as np

try:
    import concourse.bass as bass
except ImportError:  # make concourse importable in a bare grading dir
    for _p in ("/opt/trn_rl_repo", os.path.expanduser("~/.axon_site/_ro/trn_rl_repo")):
        if os.path.isdir(_p) and _p not in sys.path:
            sys.path.insert(0, _p)
    import concourse.bass as bass

from contextlib import ExitStack

import ml_dtypes

import concourse.mybir as mybir
import concourse.tile as tile
from concourse import bacc
from concourse.bass_utils import run_bass_kernel_spmd


def _setup_act_tables():
    """Pin the ACT function table to the set that covers exp+ln+identity+copy
    so the kernel never reloads LUTs mid-flight."""
    import json
    import shutil
    import tempfile

    import concourse.hw_specs as hw_specs
    from concourse import bacc as _bacc

    if os.environ.get("BASS_ACT_ROOT_JSON_PATH"):
        return  # already configured
    from neuronxcc.driver.Job import Job

    orig = os.path.join(
        Job.getPackageDir(), "pwp", "pwp_bin_trainium", "act_info.json"
    )
    assert os.path.isfile(orig), orig
    dst = os.path.join(tempfile.gettempdir(), "mha_act_tables")
    if not os.path.isdir(dst):
        tmp = dst + ".tmp"
        shutil.rmtree(tmp, ignore_errors=True)
        shutil.copytree(os.path.dirname(orig), tmp)
        with open(os.path.join(tmp, "act_info.json")) as f:
            info = json.load(f)
        sets = info["act_func_sets"]
        want = [s for s in sets if s["name"] == "natural_log_exp_and_others"]
        rest = [s for s in sets if s["name"] != "natural_log_exp_and_others"]
        info["act_func_sets"] = want + rest
        with open(os.path.join(tmp, "act_info.json"), "w") as f:
            json.dump(info, f)
        os.replace(tmp, dst)
    path = os.path.join(dst, "act_info.json")
    os.environ["BASS_ACT_ROOT_JSON_PATH"] = path

    def patched(module_arch):
        with open(path) as af:
            act_info = json.load(af)
        return {
            ent["name"]: {
                mybir.ActivationFunctionType.from_pwp(v) for v in ent["act"].keys()
            }
            for ent in act_info["act_func_sets"]
        }

    hw_specs.get_activation_tables = patched
    _bacc.get_activation_tables = patched
    from concourse import bass_interp as _bi

    _bi.get_activation_tables = patched


B, S, C = 8, 1024, 1024
H, DK, DV = 16, 64, 64
P = 128
NT = 8  # number of 128-tiles along S / C / H*DK
CH = 512  # free-dim chunk (one PSUM bank of fp32)
NCH = S // CH
NPAIR = H // 2

FP = mybir.dt.float32
BF = mybir.dt.bfloat16
BF_NP = ml_dtypes.bfloat16
AFT = mybir.ActivationFunctionType
ALU = mybir.AluOpType


def build_nc() -> bass.Bass:
    _setup_act_tables()
    nc = bacc.Bacc()

    # host-packed inputs; see prep_shared/prep_core for layouts
    xq = nc.dram_tensor("xq", [NCH, P, NT, CH], BF, kind="ExternalInput")
    xk = nc.dram_tensor("xk", [NCH, P, NT, CH], BF, kind="ExternalInput")
    xv = nc.dram_tensor("xv", [NT, P, NT, P], BF, kind="ExternalInput")
    wq = nc.dram_tensor("wq", [NT, P, NT, P], BF, kind="ExternalInput")
    wk = nc.dram_tensor("wk", [NT, P, NT, P], BF, kind="ExternalInput")
    wv = nc.dram_tensor("wv", [NCH, P, NT, CH], BF, kind="ExternalInput")
    wo = nc.dram_tensor("wo", [H * DV, C], BF, kind="ExternalInput")
    bqd = nc.dram_tensor("bq", [P, NT], FP, kind="ExternalInput")
    bkd = nc.dram_tensor("bk", [P, NT], FP, kind="ExternalInput")
    bvd = nc.dram_tensor("bv", [P, H * DV], BF, kind="ExternalInput")
    bod = nc.dram_tensor("bo", [1, C], BF, kind="ExternalInput")
    y = nc.dram_tensor("y", [S, C], FP, kind="ExternalOutput")

    # binary causal mask [t,q] for the 128-wide diagonal block (1 iff t<=q)
    tri_d = nc.inline_tensor(
        np.triu(np.ones((P, P), np.float32)).astype(BF_NP), "tri"
    )

    wo_r = wo.rearrange("(ko p) c -> p ko c", p=P)
    y_r = y.rearrange("(mo p) c -> p mo c", p=P)

    with tile.TileContext(nc) as tc, ExitStack() as octx:
        const = octx.enter_context(tc.tile_pool(name="const", bufs=1))
        big = octx.enter_context(tc.tile_pool(name="big", bufs=1))
        wkp = octx.enter_context(tc.tile_pool(name="wkp", bufs=3))
        wqp = octx.enter_context(tc.tile_pool(name="wqp", bufs=3))
        xvp = octx.enter_context(tc.tile_pool(name="xvp", bufs=3))
        pst = octx.enter_context(tc.tile_pool(name="pst", bufs=4, space="PSUM"))
        ppo = octx.enter_context(tc.tile_pool(name="ppo", bufs=2, space="PSUM"))
        ppy = octx.enter_context(tc.tile_pool(name="ppy", bufs=2, space="PSUM"))
        pchp = octx.enter_context(tc.tile_pool(name="pchp", bufs=6))
        rrp = octx.enter_context(tc.tile_pool(name="rrp", bufs=2))
        rrepp = octx.enter_context(tc.tile_pool(name="rrepp", bufs=2))
        ytp = octx.enter_context(tc.tile_pool(name="ytp", bufs=2))

        # ---------- constants ----------
        tri_sb = const.tile([P, P], BF, tag="tri")
        nc.sync.dma_start(tri_sb, tri_d[:])
        bq_sb = const.tile([P, NT], FP, tag="bq")
        nc.sync.dma_start(bq_sb, bqd[:])
        bk_sb = const.tile([P, NT], FP, tag="bk")
        nc.sync.dma_start(bk_sb, bkd[:])
        bv_sb = const.tile([P, H * DV], BF, tag="bv")
        nc.sync.dma_start(bv_sb, bvd[:])
        bo_sb = const.tile([1, C], BF, tag="bo")
        nc.sync.dma_start(bo_sb, bod[:])
        borep_sb = const.tile([P, C], BF, tag="borep")
        nc.gpsimd.partition_broadcast(borep_sb, bo_sb)
        # warm the ACT exp table during the DMA lead-in
        warm_sb = const.tile([1, P], FP, tag="warm")
        nc.scalar.activation(warm_sb, tri_sb[0:1, :], AFT.Exp)

        # ---------- big resident tensors ----------
        qT_sb = big.tile([P, NT, S], BF, tag="qT")
        kT2_sb = big.tile([P, H, S], BF, tag="kT2")
        v_sb = big.tile([P, NT, H, DV + 1], BF, tag="v")
        oT_sb = big.tile([P, NT, S], BF, tag="oT")
        wo_sb = big.tile([P, NT, C], BF, tag="wo")
        xq_sb = big.tile([P, NT, S], BF, tag="xq")
        xk_sb = big.tile([P, NT, S], BF, tag="xk")
        wv_sb = big.tile([P, NT, H * DV], BF, tag="wv")

        # ones column 64 -> AV matmul row 64 is the softmax denominator
        nc.vector.memset(v_sb[:, :, :, DV], 1.0)
        # zero the unused half of each head's K^T tile (zero lhsT rows nullify
        # the other head's Q rows in the packed 128-contraction)
        for h in range(H):
            hz = DK if h % 2 == 0 else 0
            nc.vector.memset(kT2_sb[hz : hz + DK, h, :], 0.0)

        # ---------- streamed-DMA tiles ----------
        wk_tiles: dict = {}
        wq_tiles: dict = {}
        xv_tiles: dict = {}

        def dma_wk(m, pool, tag="w"):
            t = pool.tile([P, NT, P], BF, tag=tag, name=f"wk{m}")
            nc.sync.dma_start(t, wk[m])
            wk_tiles[m] = t

        def dma_wq(m, pool, tag="w"):
            t = pool.tile([P, NT, P], BF, tag=tag, name=f"wq{m}")
            nc.sync.dma_start(t, wq[m])
            wq_tiles[m] = t

        def dma_xv(m, pool, tag="w"):
            t = pool.tile([P, NT, P], BF, tag=tag, name=f"xv{m}")
            nc.sync.dma_start(t, xv[m])
            xv_tiles[m] = t

        def dma_xhalf(x_sb, x_d, n, eng=None):
            eng = eng if eng is not None else nc.sync
            for h in (0, 1):
                eng.dma_start(
                    x_sb[:, 4 * h : 4 * h + 4, n * CH : (n + 1) * CH],
                    x_d[n][:, 4 * h : 4 * h + 4],
                )

        def dma_wvhalf(n, eng=None):
            eng = eng if eng is not None else nc.sync
            eng.dma_start(wv_sb[:, :, n * CH : (n + 1) * CH], wv[n])

        def dma_wo(kc_lo, kc_hi):
            nc.sync.dma_start(wo_sb[:, kc_lo:kc_hi], wo_r[:, kc_lo:kc_hi])

        # ---------- projection / output matmul groups (~1.7us PE each) ----------
        def k_group(m, n):
            w = wk_tiles[m]
            py = ppy.tile([P, CH], FP, tag="py", name=f"pk{m}{n}")
            for kc in range(NT):
                nc.tensor.matmul(
                    py,
                    w[:, kc, :],
                    xk_sb[:, kc, n * CH : (n + 1) * CH],
                    start=(kc == 0),
                    stop=(kc == NT - 1),
                )
            sl = slice(n * CH, (n + 1) * CH)
            nc.scalar.activation(
                kT2_sb[0:DK, 2 * m, sl], py[0:DK], AFT.Identity,
                bias=bk_sb[0:DK, m : m + 1],
            )
            nc.scalar.activation(
                kT2_sb[DK:P, 2 * m + 1, sl], py[DK:P], AFT.Identity,
                bias=bk_sb[DK:P, m : m + 1],
            )

        def q_group(m, n, dve_evac=False):
            w = wq_tiles[m]
            py = ppy.tile([P, CH], FP, tag="py", name=f"pq{m}{n}")
            for kc in range(NT):
                nc.tensor.matmul(
                    py,
                    w[:, kc, :],
                    xq_sb[:, kc, n * CH : (n + 1) * CH],
                    start=(kc == 0),
                    stop=(kc == NT - 1),
                )
            if dve_evac:
                nc.vector.tensor_scalar_add(
                    qT_sb[:, m, n * CH : (n + 1) * CH], py, bq_sb[:, m : m + 1]
                )
            else:
                nc.scalar.activation(
                    qT_sb[:, m, n * CH : (n + 1) * CH], py, AFT.Identity,
                    bias=bq_sb[:, m : m + 1],
                )

        def v_group(m, n):
            xvt = xv_tiles[m]
            py = ppy.tile([P, CH], FP, tag="py", name=f"pv{m}{n}")
            for kc in range(NT):
                nc.tensor.matmul(
                    py,
                    xvt[:, kc, :],
                    wv_sb[:, kc, n * CH : (n + 1) * CH],
                    start=(kc == 0),
                    stop=(kc == NT - 1),
                )
            dst = v_sb[:, m, 8 * n : 8 * (n + 1), 0:DV]
            nc.vector.tensor_tensor(
                dst,
                py.rearrange("p (h v) -> p h v", v=DV),
                bv_sb[:, n * CH : (n + 1) * CH].rearrange("p (h v) -> p h v", v=DV),
                ALU.add,
            )

        def o_group(m, n):
            py = ppy.tile([P, CH], FP, tag="py", name=f"py{m}{n}")
            for kc in range(NT):
                nc.tensor.matmul(
                    py,
                    oT_sb[:, kc, m * P : (m + 1) * P],
                    wo_sb[:, kc, n * CH : (n + 1) * CH],
                    start=(kc == 0),
                    stop=(kc == NT - 1),
                )
            yt = ytp.tile([P, CH], FP, tag="y", name=f"yt{m}{n}")
            nc.vector.tensor_tensor(
                yt, py, borep_sb[:, n * CH : (n + 1) * CH], ALU.add
            )
            nc.sync.dma_start(y_r[:, m, n * CH : (n + 1) * CH], yt)

        # ---------- work queue ----------
        # All projection/output groups flow through one FIFO; attention rounds
        # pull items as PE fillers while the ACT engine chews on exp.
        queue: list = []
        qpos = [0]

        def pull(fence=None):
            limit = len(queue) if fence is None else fence
            if qpos[0] < limit:
                queue[qpos[0]]()
                qpos[0] += 1
                return True
            return False

        def drain(upto):
            while qpos[0] < upto:
                pull()

        # ---------- attention round for one head pair / q-chunk ----------
        def attn(hp, jc, max_fill, fence=None):
            i_list = list(range(4 * jc + 4))
            n_i = len(i_list)
            lag = 3  # AV trails St by `lag` i-steps to hide exp latency
            pos = {}
            pchs = {s: {} for s in (0, 1)}
            offs = {}
            fills = [0]

            def fill():
                # never pull past `fence`: items beyond the next unit's marker
                # may depend on THIS (or a later) attention round's oT writes
                if fills[0] < max_fill and pull(fence):
                    fills[0] += 1

            def st_step(i):
                off = max(0, i * P - jc * CH)
                w = CH - off
                offs[i] = off
                for sub in (0, 1):
                    h = 2 * hp + sub
                    stt = pst.tile(
                        [P, CH], FP, tag="st", name=f"st{hp}_{jc}_{i}_{sub}"
                    )[:, :w]
                    nc.tensor.matmul(
                        stt,
                        kT2_sb[:, h, i * P : (i + 1) * P],
                        qT_sb[:, hp, jc * CH + off : (jc + 1) * CH],
                        start=True,
                        stop=True,
                    )
                    pch = pchp.tile(
                        [P, CH], BF, tag="p", name=f"p{hp}_{jc}_{i}_{sub}"
                    )[:, :w]
                    nc.scalar.activation(pch, stt, AFT.Exp)
                    if i * P >= jc * CH:
                        nc.vector.tensor_tensor(
                            pch[:, 0:P], pch[:, 0:P], tri_sb, ALU.mult
                        )
                    pchs[sub][i] = pch

            def av_step(i):
                off = offs[i]
                for sub in (0, 1):
                    h = 2 * hp + sub
                    if i == 0:
                        pos[sub] = ppo.tile(
                            [P, CH], FP, tag="o", name=f"o{hp}_{jc}_{sub}"
                        )[: DV + 1]
                    nc.tensor.matmul(
                        pos[sub][:, off:],
                        v_sb[:, i, h, :],
                        pchs[sub][i],
                        start=(i == 0),
                        stop=(i == n_i - 1),
                    )

            for i in i_list[:lag]:
                st_step(i)
                if i >= 1:
                    fill()
            for i in i_list[lag:]:
                st_step(i)
                fill()
                av_step(i - lag)
            for i in i_list[n_i - lag :]:
                av_step(i)
                fill()

            for sub in (0, 1):
                po = pos[sub]
                # reciprocal_approx_fast only works at partition base 0, so
                # stage the denominator row 64 -> 0 with a stock copy first
                rc = rrp.tile([1, CH], FP, tag="rc", name=f"rc{hp}_{jc}_{sub}")
                nc.scalar.activation(rc, po[DV : DV + 1, :], AFT.Copy)
                rr = rrp.tile([1, CH], FP, tag="rr", name=f"rr{hp}_{jc}_{sub}")
                nc.vector.reciprocal_approx_fast(rr, rc)
                rrep = rrepp.tile(
                    [DV, CH], FP, tag="rrep", name=f"rrep{hp}_{jc}_{sub}"
                )
                nc.gpsimd.partition_broadcast(rrep, rr)
                nc.vector.tensor_tensor(
                    oT_sb[sub * DV : (sub + 1) * DV, hp, jc * CH : (jc + 1) * CH],
                    po[0:DV],
                    rrep,
                    ALU.mult,
                )

        # ================= schedule =================
        # lead-in DMAs, spread across idle engine queues for parallelism
        dma_wk(0, const, "wk0")
        dma_xhalf(xk_sb, xk, 0, nc.gpsimd)
        dma_wq(0, const, "wq0")
        dma_xhalf(xq_sb, xq, 0, nc.scalar)
        for m in range(4):
            dma_xv(m, const, f"xv{m}")
        dma_wvhalf(0)
        dma_wk(1, wkp)
        dma_xhalf(xk_sb, xk, 1, nc.gpsimd)

        def KG(m, n):
            return lambda: k_group(m, n)

        def QG(m, n):
            return lambda: q_group(m, n, dve_evac=(n == 1))

        def VG(m, n):
            return lambda: v_group(m, n)

        def OG(m, n):
            return lambda: o_group(m, n)

        # units: (attn args, groups due before it, dma prefetch at drain time)
        units = [
            ((0, 0, 2), [KG(0, 0), QG(0, 0), VG(0, 0), VG(1, 0), VG(2, 0), VG(3, 0)],
             lambda: (dma_wq(1, wqp), dma_wvhalf(1))),
            ((1, 0, 2), [KG(1, 0), KG(1, 1), QG(1, 0), VG(0, 1)],
             lambda: (dma_wk(2, wkp), dma_wq(2, wqp))),
            ((2, 0, 2), [KG(2, 0), KG(2, 1), QG(2, 0), VG(1, 1)],
             lambda: (dma_wk(3, wkp), dma_wq(3, wqp))),
            ((3, 0, 2), [KG(3, 0), KG(3, 1), QG(3, 0), VG(2, 1)],
             lambda: (dma_wk(4, wkp), dma_wq(4, wqp))),
            ((4, 0, 2), [VG(3, 1), KG(4, 0), KG(4, 1), QG(4, 0)],
             lambda: (dma_wk(5, wkp), dma_wq(5, wqp), dma_xv(4, xvp))),
            ((5, 0, 2), [KG(5, 0), KG(5, 1), QG(5, 0), VG(4, 0), VG(4, 1)],
             lambda: (dma_wk(6, wkp), dma_wq(6, wqp), dma_xv(5, xvp))),
            ((6, 0, 2), [KG(6, 0), KG(6, 1), QG(6, 0), VG(5, 0), VG(5, 1)],
             lambda: (dma_wk(7, wkp), dma_wq(7, wqp), dma_xv(6, xvp),
                      dma_wq(1, wqp))),
            ((7, 0, 2), [KG(7, 0), KG(7, 1), QG(7, 0), VG(6, 0), VG(6, 1)],
             lambda: (dma_xv(7, xvp), dma_xhalf(xq_sb, xq, 1),
                      dma_wq(2, wqp), dma_wo(0, 4))),
            ((0, 1, 3), [KG(0, 1), VG(7, 0), VG(7, 1), QG(0, 1), QG(1, 1)],
             lambda: (dma_wq(3, wqp), dma_wo(4, 8))),
            ((1, 1, 3), [QG(2, 1), OG(0, 0)],
             lambda: dma_wq(4, wqp)),
            ((2, 1, 3), [QG(3, 1), OG(0, 1)],
             lambda: dma_wq(5, wqp)),
            ((3, 1, 3), [QG(4, 1), OG(1, 0)],
             lambda: dma_wq(6, wqp)),
            ((4, 1, 3), [QG(5, 1), OG(1, 1)],
             lambda: dma_wq(7, wqp)),
            ((5, 1, 3), [QG(6, 1), OG(2, 0)], None),
            ((6, 1, 3), [QG(7, 1), OG(2, 1)], None),
            ((7, 1, 3), [OG(3, 0), OG(3, 1)], None),
        ]
        tail = []
        for m in range(4, NT):
            tail.append(OG(m, 0))
            tail.append(OG(m, 1))

        markers = []
        for _, groups, _ in units:
            queue.extend(groups)
            markers.append(len(queue))
        queue.extend(tail)

        fences = markers[1:] + [markers[-1]]
        for u, (args, _, dma_fn) in enumerate(units):
            if dma_fn is not None:
                dma_fn()
            drain(markers[u])
            attn(*args, fence=fences[u])
        drain(len(queue))

    nc.finalize()
    return nc


_NC_CACHE = None


def _get_nc() -> bass.Bass:
    global _NC_CACHE
    if _NC_CACHE is None:
        _NC_CACHE = build_nc()
    return _NC_CACHE


def prep_shared(Wq, bq, Wk, bk, Wv, bv, Wo, bo):
    """Host-side packing of weights/biases (shared by all cores)."""
    scale = 1.0 / math.sqrt(DK)
    Wq = np.asarray(Wq, np.float32)
    Wk = np.asarray(Wk, np.float32)
    Wv = np.asarray(Wv, np.float32)
    Wo = np.asarray(Wo, np.float32)
    WqT = (Wq.transpose(1, 0, 2).reshape(C, H * DK) * scale).astype(BF_NP)
    WkT = Wk.transpose(1, 0, 2).reshape(C, H * DK).astype(BF_NP)
    WvT = Wv.transpose(1, 0, 2).reshape(C, H * DV).astype(BF_NP)
    out = {
        # [m, p, kc, mi] <- W[kc*128+p, m*128+mi]
        "wq": np.ascontiguousarray(
            WqT.reshape(NT, P, NT, P).transpose(2, 1, 0, 3)
        ),
        "wk": np.ascontiguousarray(
            WkT.reshape(NT, P, NT, P).transpose(2, 1, 0, 3)
        ),
        # [n, p, kc, j] <- W[kc*128+p, n*512+j]
        "wv": np.ascontiguousarray(
            WvT.reshape(NT, P, NCH, CH).transpose(2, 1, 0, 3)
        ),
        "wo": Wo.astype(BF_NP),
        "bq": np.ascontiguousarray(
            (np.asarray(bq, np.float32).reshape(H * DK) * scale)
            .reshape(NT, P)
            .T.astype(np.float32)
        ),
        "bk": np.ascontiguousarray(
            np.asarray(bk, np.float32).reshape(NT, P).T.astype(np.float32)
        ),
        "bv": np.ascontiguousarray(
            np.broadcast_to(
                np.asarray(bv, np.float32).reshape(1, H * DV), (P, H * DV)
            ).astype(BF_NP)
        ),
        "bo": np.ascontiguousarray(
            np.asarray(bo, np.float32).reshape(1, C).astype(BF_NP)
        ),
    }
    return out


def prep_core(q_embs_b, k_embs_b, v_embs_b):
    XqT = np.asarray(q_embs_b, np.float32).T.astype(BF_NP)  # [C, S]
    XkT = np.asarray(k_embs_b, np.float32).T.astype(BF_NP)
    XvT = np.asarray(v_embs_b, np.float32).T.astype(BF_NP)
    return {
        # [n, p, kc, j] <- X^T[kc*128+p, n*512+j]
        "xq": np.ascontiguousarray(
            XqT.reshape(NT, P, NCH, CH).transpose(2, 1, 0, 3)
        ),
        "xk": np.ascontiguousarray(
            XkT.reshape(NT, P, NCH, CH).transpose(2, 1, 0, 3)
        ),
        # [m, p, kc, mi] <- X^T[kc*128+p, m*128+mi]
        "xv": np.ascontiguousarray(
            XvT.reshape(NT, P, NT, P).transpose(2, 1, 0, 3)
        ),
    }


def kernel(q_embs, k_embs, v_embs, Wq, bq, Wk, bk, Wv, bv, Wo, bo, **run_kwargs):
    nc = _get_nc()
    shared = prep_shared(Wq, bq, Wk, bk, Wv, bv, Wo, bo)
    q_embs = np.asarray(q_embs, np.float32)
    k_embs = np.asarray(k_embs, np.float32)
    v_embs = np.asarray(v_embs, np.float32)
    in_maps = []
    for b in range(B):
        m = dict(shared)
        m.update(prep_core(q_embs[b], k_embs[b], v_embs[b]))
        in_maps.append(m)
    res = run_bass_kernel_spmd(nc, in_maps, core_ids=list(range(B)), **run_kwargs)
    out = np.stack([res.results[i]["y"] for i in range(B)], axis=0)
    if run_kwargs:
        kernel.last_results = res
    return out


if __name__ == "__main__":
    rng = np.random.default_rng(0)
    inputs = {
        "q_embs": rng.standard_normal((B, S, C), np.float32),
        "k_embs": rng.standard_normal((B, S, C), np.float32),
        "v_embs": rng.standard_normal((B, S, C), np.float32),
        "Wq": rng.standard_normal((H, C, DK), np.float32) * 0.02,
        "bq": np.zeros((H, DK), np.float32),
        "Wk": rng.standard_normal((H, C, DK), np.float32) * 0.02,
        "bk": np.zeros((H, DK), np.float32),
        "Wv": rng.standard_normal((H, C, DV), np.float32) * 0.02,
        "bv": np.zeros((H, DV), np.float32),
        "Wo": rng.standard_normal((H * DV, C), np.float32) * 0.02,
        "bo": np.zeros((C,), np.float32),
    }
    out = kernel(**inputs)
    print(out.shape, out.dtype)
